# revision 10
# baseline (speedup 1.0000x reference)
"""Expert-parallel grouped-MLP (MoE experts) kernel for 8 Trainium2 cores.

Problem: y = W2_e @ silu(W1_e @ x_e + b1_e) + b2_e for E=16 independent
experts (grouped 1x1 conv), B=8 batches, C=256 channels/expert, CAP=4,
L=1024 positions.

Sharding: expert-parallel — core i owns experts {2i, 2i+1}; no cross-core
communication. Host pre-transposes weights into lhsT layout and pre-casts
x/W1/W2 to fp16 (same numerics as an on-device DVE cast, but half the
DMA bytes and no cast on the startup critical path).

Per (b, e) pair on-device:
  layer 1: 8 m-tiles x (2k x 2n) fp16 matmuls -> PSUM[128,1024]
           ACT silu(. + b1) PSUM -> h SBUF [128, 8x1024] (fp16)
  layer 2: 2 j-tiles x (8q x 2n) accumulating matmuls -> PSUM[128,1024]
           DVE + b2 PSUM -> y SBUF fp32, DMA out

DMA-trigger scheduling: a dma_start costs ~0.6-0.8us of sequencer time,
so triggers are coalesced into few multi-dim-AP descriptors (one DMA per
expert for W1/W2, one per pair for x, one per bias tensor) and split
across queues: GpSimd carries all x loads (first trigger ~6.5us -> pair-0
x complete ~8.5us), Sync carries weights/biases and the y stores
(gpsimd-SWDGE stores lengthened the teardown by ~3us when tried).
"""
import numpy as np

import concourse.tile as tile
from concourse import bacc, mybir
from concourse.bass_utils import run_bass_kernel_spmd

# Problem constants (hardcoded per contract)
B, E, C, CAP, L = 8, 16, 256, 4, 1024
F = C * CAP            # 1024 hidden per expert
NCORES = 8
EPC = E // NCORES      # 2 experts per core
P = 128                # partitions
KT = C // P            # 2 k-tiles (layer-1 contraction)
MT = F // P            # 8 m-tiles (layer-1 output partitions)
JT = C // P            # 2 j-tiles (layer-2 output partitions)
QT = F // P            # 8 q-tiles (layer-2 contraction)
NT = L // 512          # 2 n-tiles of 512 cols
N_WARMUP = 5           # dummy PE warmup matmuls (bridge engine-boot -> data)

_FP32 = mybir.dt.float32
_FP16 = mybir.dt.float16


def _build():
    nc = bacc.Bacc("TRN2", target_bir_lowering=False, debug=False)

    # All inputs are host-prearranged into SBUF layout so every input DMA
    # is a contiguous [128, N] copy (descriptor processing cost on the
    # trigger queue scales with segment count — a gathered bias AP cost
    # 7.7us of sync-queue time when tried).
    xs_d = nc.dram_tensor("xs", [B, EPC, P, KT * L], _FP16, kind="ExternalInput")
    w1t_d = nc.dram_tensor("w1t", [EPC, C, F], _FP16, kind="ExternalInput")
    b1s_d = nc.dram_tensor("b1s", [P, EPC * MT], _FP32, kind="ExternalInput")
    w2t_d = nc.dram_tensor("w2t", [EPC, P, QT * C], _FP16, kind="ExternalInput")
    b2s_d = nc.dram_tensor("b2s", [P, EPC * JT], _FP32, kind="ExternalInput")
    ys_d = nc.dram_tensor("ys", [B, EPC * C, L], _FP32, kind="ExternalOutput")

    with tile.TileContext(nc) as tc:
        with (
            tc.tile_pool(name="const", bufs=1) as cpool,
            tc.tile_pool(name="x", bufs=6) as xpool,
            tc.tile_pool(name="h", bufs=2) as hpool,
            tc.tile_pool(name="y", bufs=3) as ypool,
            tc.tile_pool(name="ps", bufs=4, space="PSUM") as pspool,
        ):
            # ---- PE warmup: zero bf16 matmuls; memsets on Vector (the
            # GpSimd queue is reserved for the x-load DMA triggers) ----
            wdum = cpool.tile([P, P], mybir.dt.bfloat16, tag="wdum")
            rdum = cpool.tile([P, 512], mybir.dt.bfloat16, tag="rdum")
            bdum = cpool.tile([P, 1], _FP32, tag="bdum")
            nc.vector.memset(wdum[:], 0.0)
            nc.vector.memset(rdum[:], 0.0)
            nc.vector.memset(bdum[:], 0.0)
            actdum = cpool.tile([P, 1], _FP32, tag="actdum")
            # AP bias (not immediate) so the ACT table config matches the
            # real silus and the table is not reloaded mid-stream
            nc.scalar.activation(actdum[:], rdum[:, :1],
                                 mybir.ActivationFunctionType.Silu,
                                 bias=bdum[:, 0:1])
            for i in range(N_WARMUP):
                pdum = pspool.tile([P, L], _FP32, tag="ps")
                nc.tensor.matmul(pdum[:, :512], wdum[:], rdum[:],
                                 start=True, stop=True)

            # ---- weight/bias tiles (fp16 direct from DRAM, no casts) ----
            # w1sb[e][k]: [128, F];   [p, f] = W1T[e, k*128+p, f]
            # w2sb[e]:    [128, QT*C]; [p, q*C+c] = W2T[e, q*128+p, c]
            w1sb = [[cpool.tile([P, F], _FP16, tag=f"w1_{e}_{k}",
                                name=f"w1sb_{e}_{k}")
                     for k in range(KT)] for e in range(EPC)]
            w2sb = [cpool.tile([P, QT * C], _FP16, tag=f"w2_{e}",
                               name=f"w2sb_{e}")
                    for e in range(EPC)]
            b1sb = cpool.tile([P, EPC * MT], _FP32, tag="b1")  # col e*MT+m
            b2sb = cpool.tile([P, EPC * JT], _FP32, tag="b2")  # col e*JT+j

            def load_w1(e, k, eng):
                eng.dma_start(
                    w1sb[e][k][:],
                    w1t_d.ap()[e, k * P:(k + 1) * P, :],
                )

            def load_w2(e, eng):
                # one contiguous DMA per expert (host-prearranged layout)
                eng.dma_start(w2sb[e][:], w2t_d.ap()[e])

            def load_biases(eng):
                eng.dma_start(b1sb[:], b1s_d.ap()[:, :])
                eng.dma_start(b2sb[:], b2s_d.ap()[:, :])

            def load_x(b, e):
                # one contiguous 512KB DMA per pair (host-prearranged)
                xt = xpool.tile([P, KT * L], _FP16, tag="x",
                                name=f"x_{b}_{e}")
                nc.gpsimd.dma_start(xt[:], xs_d.ap()[b, e])
                return xt

            # ---- startup-critical DMAs ----
            # ALL pair-0 data on the sync queue: sync-triggered DMAs ride
            # the hardware DGE rings and land ~2-4us sooner than
            # gpsimd-SWDGE ones. x(0,0) as two k-half DMAs interleaved
            # with the W1 tiles in consumption order.
            x00 = [cpool.tile([P, L], _FP16, tag=f"x00_{k}",
                              name=f"x00_{k}") for k in range(KT)]
            nc.sync.dma_start(x00[0][:], xs_d.ap()[0, 0, :, 0:L])
            load_w1(0, 0, nc.sync)
            nc.sync.dma_start(x00[1][:], xs_d.ap()[0, 0, :, L:2 * L])
            load_w1(0, 1, nc.sync)
            # keep the sync queue SHORT before the first matmuls; biases
            # and W2 ride the scalar queue (idle until the first silu)
            load_biases(nc.scalar)
            load_w2(0, nc.scalar)

            # ---- per-(expert, batch) pipeline ----
            for e in range(EPC):
                for b in range(B):
                    first_pair = (e == 0 and b == 0)
                    last_pair = (e == EPC - 1 and b == B - 1)
                    xsb = None if first_pair else load_x(b, e)
                    if e == 0 and b == 1:
                        # expert-1 weights (needed at pair 8, ~120us in);
                        # sync queue is idle from here on
                        load_w1(1, 0, nc.sync)
                        load_w1(1, 1, nc.sync)
                        load_w2(1, nc.sync)

                    # layer 1: h = silu(W1 @ x + b1), h[p, m*L + l]
                    hsb = hpool.tile([P, MT * L], _FP16, tag="h")
                    for m in range(MT):
                        psh = pspool.tile([P, L], _FP32, tag="ps")
                        for k in range(KT):
                            rhs_t = x00[k] if first_pair else xsb
                            off = 0 if first_pair else k * L
                            for n in range(NT):
                                nc.tensor.matmul(
                                    psh[:, n * 512:(n + 1) * 512],
                                    w1sb[e][k][:, m * P:(m + 1) * P],
                                    rhs_t[:, off + n * 512:
                                            off + (n + 1) * 512],
                                    start=(k == 0),
                                    stop=(k == KT - 1),
                                )
                        nc.scalar.activation(
                            hsb[:, m * L:(m + 1) * L],
                            psh[:],
                            mybir.ActivationFunctionType.Silu,
                            bias=b1sb[:, e * MT + m: e * MT + m + 1],
                        )

                    # layer 2: y = W2 @ h + b2
                    for j in range(JT):
                        if last_pair:
                            # separate 1-bank psum per n so DVE(n0) overlaps
                            # the n1 matmul chain; store triggered from the
                            # Vector queue to keep the tail short
                            for n in range(NT):
                                psn = pspool.tile([P, 512], _FP32, tag="ps",
                                                  name=f"psn_{j}_{n}")
                                for q in range(QT):
                                    nc.tensor.matmul(
                                        psn[:],
                                        w2sb[e][:, q * C + j * P:
                                                q * C + (j + 1) * P],
                                        hsb[:, q * L + n * 512:
                                              q * L + (n + 1) * 512],
                                        start=(q == 0),
                                        stop=(q == QT - 1),
                                    )
                                ysn = ypool.tile([P, 512], _FP32, tag="y",
                                                 name=f"ysn_{j}_{n}")
                                nc.vector.tensor_scalar_add(
                                    ysn[:],
                                    psn[:],
                                    b2sb[:, e * JT + j: e * JT + j + 1],
                                )
                                nc.sync.dma_start(
                                    ys_d.ap()[b,
                                              e * C + j * P: e * C + (j + 1) * P,
                                              n * 512:(n + 1) * 512],
                                    ysn[:],
                                )
                            continue
                        psy = pspool.tile([P, L], _FP32, tag="ps")
                        for q in range(QT):
                            for n in range(NT):
                                nc.tensor.matmul(
                                    psy[:, n * 512:(n + 1) * 512],
                                    w2sb[e][:, q * C + j * P: q * C + (j + 1) * P],
                                    hsb[:, q * L + n * 512: q * L + (n + 1) * 512],
                                    start=(q == 0),
                                    stop=(q == QT - 1),
                                )
                        ysb = ypool.tile([P, L], _FP32, tag="y",
                                         name=f"ysb_{e}_{b}_{j}")
                        nc.vector.tensor_scalar_add(
                            ysb[:],
                            psy[:],
                            b2sb[:, e * JT + j: e * JT + j + 1],
                        )
                        nc.sync.dma_start(
                            ys_d.ap()[b, e * C + j * P: e * C + (j + 1) * P, :],
                            ysb[:],
                        )

    nc.compile()
    return nc


_NC_CACHE = None


def _get_nc():
    global _NC_CACHE
    if _NC_CACHE is None:
        _NC_CACHE = _build()
    return _NC_CACHE


def _shard_inputs(x, W1, b1, W2, b2):
    """Full inputs -> list of 8 per-core input dicts (expert-parallel).

    Host pre-casts x/W1/W2 to fp16 — identical numerics to an on-device
    DVE cast, but half the DMA bytes and no cast latency.
    """
    # x -> [B, E, P, KT*L]: [b, e, p, k*L+l] = x[b, e*C + k*128 + p, l]
    x16 = (x.astype(np.float16)
            .reshape(B, E, KT, P, L).transpose(0, 1, 3, 2, 4)
            .reshape(B, E, P, KT * L))
    # W1T[e] = W1r[e].T -> [E, C, F] (rows are k*128+p, already contiguous)
    w1t = np.ascontiguousarray(
        W1.astype(np.float32).reshape(E, F, C).transpose(0, 2, 1)
    ).astype(np.float16)
    # W2 -> [E, P, QT*C]: [e, p, q*C+c] = W2T[e, q*128+p, c]
    w2t = (W2.astype(np.float32).reshape(E, C, F).transpose(0, 2, 1)
             .reshape(E, QT, P, C).transpose(0, 2, 1, 3)
             .reshape(E, P, QT * C)).astype(np.float16)
    # biases -> [P, E*MT] / [P, E*JT]: [p, e*MT+m] = b1[e, m*128+p]
    b1r = (b1.astype(np.float32).reshape(E, MT, P)
             .transpose(2, 0, 1).reshape(P, E * MT))
    b2r = (b2.astype(np.float32).reshape(E, JT, P)
             .transpose(2, 0, 1).reshape(P, E * JT))
    in_maps = []
    for i in range(NCORES):
        es = slice(i * EPC, (i + 1) * EPC)
        b1c = b1r.reshape(P, E, MT)[:, es].reshape(P, EPC * MT)
        b2c = b2r.reshape(P, E, JT)[:, es].reshape(P, EPC * JT)
        in_maps.append({
            "xs": np.ascontiguousarray(x16[:, es]),
            "w1t": np.ascontiguousarray(w1t[es]),
            "b1s": np.ascontiguousarray(b1c),
            "w2t": np.ascontiguousarray(w2t[es]),
            "b2s": np.ascontiguousarray(b2c),
        })
    return in_maps


def run(x, W1, b1, W2, b2, trace=False, **trace_kwargs):
    nc = _get_nc()
    in_maps = _shard_inputs(x, W1, b1, W2, b2)
    res = run_bass_kernel_spmd(
        nc, in_maps, core_ids=list(range(NCORES)), trace=trace, **trace_kwargs
    )
    y = np.concatenate([res.results[i]["ys"] for i in range(NCORES)], axis=1)
    return y, res


def kernel(x, W1, b1, W2, b2):
    y, _ = run(x, W1, b1, W2, b2)
    return y.astype(np.float32)


# revision 11
# speedup vs baseline: 1.1781x; 1.1781x over previous
"""Expert-parallel grouped-MLP (MoE experts) kernel for 8 Trainium2 cores.

Problem: y = W2_e @ silu(W1_e @ x_e + b1_e) + b2_e for E=16 independent
experts (grouped 1x1 conv), B=8 batches, C=256 channels/expert, CAP=4,
L=1024 positions.

Sharding: expert-parallel — core i owns experts {2i, 2i+1}; no cross-core
communication. Host pre-transposes weights into lhsT layout and pre-casts
x/W1/W2 to fp16 (same numerics as an on-device DVE cast, but half the
DMA bytes and no cast on the startup critical path).

Per (b, e) pair on-device:
  layer 1: 8 m-tiles x (2k x 2n) fp16 matmuls -> PSUM[128,1024]
           ACT silu(. + b1) PSUM -> h SBUF [128, 8x1024] (fp16)
  layer 2: 2 j-tiles x (8q x 2n) accumulating matmuls -> PSUM[128,1024]
           DVE + b2 PSUM -> y SBUF fp32, DMA out

DMA-trigger scheduling: a dma_start costs ~0.6-0.8us of sequencer time,
so triggers are coalesced into few multi-dim-AP descriptors (one DMA per
expert for W1/W2, one per pair for x, one per bias tensor) and split
across queues: GpSimd carries all x loads (first trigger ~6.5us -> pair-0
x complete ~8.5us), Sync carries weights/biases and the y stores
(gpsimd-SWDGE stores lengthened the teardown by ~3us when tried).
"""
import numpy as np

import concourse.tile as tile
from concourse import bacc, mybir
from concourse.bass_utils import run_bass_kernel_spmd

# Problem constants (hardcoded per contract)
B, E, C, CAP, L = 8, 16, 256, 4, 1024
F = C * CAP            # 1024 hidden per expert
NCORES = 8
EPC = E // NCORES      # 2 experts per core
P = 128                # partitions
KT = C // P            # 2 k-tiles (layer-1 contraction)
MT = F // P            # 8 m-tiles (layer-1 output partitions)
JT = C // P            # 2 j-tiles (layer-2 output partitions)
QT = F // P            # 8 q-tiles (layer-2 contraction)
NT = L // 512          # 2 n-tiles of 512 cols
N_WARMUP = 5           # dummy PE warmup matmuls (bridge engine-boot -> data)

_FP32 = mybir.dt.float32
_FP16 = mybir.dt.float16


def _build():
    nc = bacc.Bacc("TRN2", target_bir_lowering=False, debug=False)

    # All inputs are host-prearranged into SBUF layout so every input DMA
    # is a contiguous [128, N] copy (descriptor processing cost on the
    # trigger queue scales with segment count — a gathered bias AP cost
    # 7.7us of sync-queue time when tried).
    xs_d = nc.dram_tensor("xs", [B, EPC, P, KT * L], _FP16, kind="ExternalInput")
    w1t_d = nc.dram_tensor("w1t", [EPC, C, F], _FP16, kind="ExternalInput")
    b1s_d = nc.dram_tensor("b1s", [P, EPC * MT], _FP32, kind="ExternalInput")
    w2t_d = nc.dram_tensor("w2t", [EPC, P, QT * C], _FP16, kind="ExternalInput")
    b2s_d = nc.dram_tensor("b2s", [P, EPC * JT], _FP32, kind="ExternalInput")
    ys_d = nc.dram_tensor("ys", [B, EPC * C, L], _FP32, kind="ExternalOutput")

    with tile.TileContext(nc) as tc:
        with (
            tc.tile_pool(name="const", bufs=1) as cpool,
            tc.tile_pool(name="x", bufs=6) as xpool,
            tc.tile_pool(name="h", bufs=2) as hpool,
            tc.tile_pool(name="y", bufs=3) as ypool,
            tc.tile_pool(name="ps", bufs=8, space="PSUM") as pspool,
        ):
            # ---- PE warmup: zero bf16 matmuls; memsets on Vector (the
            # GpSimd queue is reserved for the x-load DMA triggers) ----
            wdum = cpool.tile([P, P], mybir.dt.bfloat16, tag="wdum")
            rdum = cpool.tile([P, 512], mybir.dt.bfloat16, tag="rdum")
            bdum = cpool.tile([P, 1], _FP32, tag="bdum")
            nc.vector.memset(wdum[:], 0.0)
            nc.vector.memset(rdum[:], 0.0)
            nc.vector.memset(bdum[:], 0.0)
            actdum = cpool.tile([P, 1], _FP32, tag="actdum")
            # AP bias (not immediate) so the ACT table config matches the
            # real silus and the table is not reloaded mid-stream
            nc.scalar.activation(actdum[:], rdum[:, :1],
                                 mybir.ActivationFunctionType.Silu,
                                 bias=bdum[:, 0:1])
            for i in range(N_WARMUP):
                pdum = pspool.tile([P, 512], _FP32, tag="ps")
                nc.tensor.matmul(pdum[:], wdum[:], rdum[:],
                                 start=True, stop=True)

            # ---- weight/bias tiles (fp16 direct from DRAM, no casts) ----
            # w1sb[e][k]: [128, F];   [p, f] = W1T[e, k*128+p, f]
            # w2sb[e]:    [128, QT*C]; [p, q*C+c] = W2T[e, q*128+p, c]
            w1sb = [[cpool.tile([P, F], _FP16, tag=f"w1_{e}_{k}",
                                name=f"w1sb_{e}_{k}")
                     for k in range(KT)] for e in range(EPC)]
            w2sb = [cpool.tile([P, QT * C], _FP16, tag=f"w2_{e}",
                               name=f"w2sb_{e}")
                    for e in range(EPC)]
            b1sb = cpool.tile([P, EPC * MT], _FP32, tag="b1")  # col e*MT+m
            b2sb = cpool.tile([P, EPC * JT], _FP32, tag="b2")  # col e*JT+j

            def load_w1(e, k, eng):
                eng.dma_start(
                    w1sb[e][k][:],
                    w1t_d.ap()[e, k * P:(k + 1) * P, :],
                )

            def load_w2(e, eng):
                # one contiguous DMA per expert (host-prearranged layout)
                eng.dma_start(w2sb[e][:], w2t_d.ap()[e])

            def load_biases(eng):
                eng.dma_start(b1sb[:], b1s_d.ap()[:, :])
                eng.dma_start(b2sb[:], b2s_d.ap()[:, :])

            def load_x(b, e):
                # one contiguous 512KB DMA per pair (host-prearranged)
                xt = xpool.tile([P, KT * L], _FP16, tag="x",
                                name=f"x_{b}_{e}")
                nc.gpsimd.dma_start(xt[:], xs_d.ap()[b, e])
                return xt

            # ---- startup-critical DMAs ----
            # ALL pair-0 data on the sync queue: sync-triggered DMAs ride
            # the hardware DGE rings and land ~2-4us sooner than
            # gpsimd-SWDGE ones. x(0,0) as two k-half DMAs interleaved
            # with the W1 tiles in consumption order.
            x00 = [cpool.tile([P, L], _FP16, tag=f"x00_{k}",
                              name=f"x00_{k}") for k in range(KT)]
            nc.sync.dma_start(x00[0][:], xs_d.ap()[0, 0, :, 0:L])
            load_w1(0, 0, nc.sync)
            nc.sync.dma_start(x00[1][:], xs_d.ap()[0, 0, :, L:2 * L])
            load_w1(0, 1, nc.sync)
            # keep the sync queue SHORT before the first matmuls; biases
            # and W2 ride the scalar queue (idle until the first silu)
            load_biases(nc.scalar)
            load_w2(0, nc.scalar)

            # ---- per-(expert, batch) pipeline ----
            for e in range(EPC):
                for b in range(B):
                    first_pair = (e == 0 and b == 0)
                    last_pair = (e == EPC - 1 and b == B - 1)
                    xsb = None if first_pair else load_x(b, e)
                    if e == 0 and b == 1:
                        # expert-1 weights (needed at pair 8, ~120us in);
                        # sync queue is idle from here on
                        load_w1(1, 0, nc.sync)
                        load_w1(1, 1, nc.sync)
                        load_w2(1, nc.sync)

                    # layer 1: h = silu(W1 @ x + b1), h[p, m*L + l]
                    hsb = hpool.tile([P, MT * L], _FP16, tag="h")
                    for m in range(MT):
                        # two 1-bank psum tiles per m-tile: finer slot
                        # recycling so the m-loop never stalls on silus
                        psh = [pspool.tile([P, 512], _FP32, tag="ps",
                                           name=f"psh_{e}_{b}_{m}_{n}")
                               for n in range(NT)]
                        for k in range(KT):
                            rhs_t = x00[k] if first_pair else xsb
                            off = 0 if first_pair else k * L
                            for n in range(NT):
                                nc.tensor.matmul(
                                    psh[n][:],
                                    w1sb[e][k][:, m * P:(m + 1) * P],
                                    rhs_t[:, off + n * 512:
                                            off + (n + 1) * 512],
                                    start=(k == 0),
                                    stop=(k == KT - 1),
                                )
                        for n in range(NT):
                            nc.scalar.activation(
                                hsb[:, m * L + n * 512:
                                      m * L + (n + 1) * 512],
                                psh[n][:],
                                mybir.ActivationFunctionType.Silu,
                                bias=b1sb[:, e * MT + m: e * MT + m + 1],
                            )

                    # layer 2: y = W2 @ h + b2
                    for j in range(JT):
                        if last_pair:
                            # separate 1-bank psum per n so DVE(n0) overlaps
                            # the n1 matmul chain; store triggered from the
                            # Vector queue to keep the tail short
                            for n in range(NT):
                                psn = pspool.tile([P, 512], _FP32, tag="ps",
                                                  name=f"psn_{j}_{n}")
                                for q in range(QT):
                                    nc.tensor.matmul(
                                        psn[:],
                                        w2sb[e][:, q * C + j * P:
                                                q * C + (j + 1) * P],
                                        hsb[:, q * L + n * 512:
                                              q * L + (n + 1) * 512],
                                        start=(q == 0),
                                        stop=(q == QT - 1),
                                    )
                                ysn = ypool.tile([P, 512], _FP32, tag="y",
                                                 name=f"ysn_{j}_{n}")
                                nc.vector.tensor_scalar_add(
                                    ysn[:],
                                    psn[:],
                                    b2sb[:, e * JT + j: e * JT + j + 1],
                                )
                                nc.sync.dma_start(
                                    ys_d.ap()[b,
                                              e * C + j * P: e * C + (j + 1) * P,
                                              n * 512:(n + 1) * 512],
                                    ysn[:],
                                )
                            continue
                        psy = [pspool.tile([P, 512], _FP32, tag="ps",
                                           name=f"psy_{e}_{b}_{j}_{n}")
                               for n in range(NT)]
                        for q in range(QT):
                            for n in range(NT):
                                nc.tensor.matmul(
                                    psy[n][:],
                                    w2sb[e][:, q * C + j * P: q * C + (j + 1) * P],
                                    hsb[:, q * L + n * 512: q * L + (n + 1) * 512],
                                    start=(q == 0),
                                    stop=(q == QT - 1),
                                )
                        ysb = ypool.tile([P, L], _FP32, tag="y",
                                         name=f"ysb_{e}_{b}_{j}")
                        for n in range(NT):
                            nc.vector.tensor_scalar_add(
                                ysb[:, n * 512:(n + 1) * 512],
                                psy[n][:],
                                b2sb[:, e * JT + j: e * JT + j + 1],
                            )
                        nc.sync.dma_start(
                            ys_d.ap()[b, e * C + j * P: e * C + (j + 1) * P, :],
                            ysb[:],
                        )

    nc.compile()
    return nc


_NC_CACHE = None


def _get_nc():
    global _NC_CACHE
    if _NC_CACHE is None:
        _NC_CACHE = _build()
    return _NC_CACHE


def _shard_inputs(x, W1, b1, W2, b2):
    """Full inputs -> list of 8 per-core input dicts (expert-parallel).

    Host pre-casts x/W1/W2 to fp16 — identical numerics to an on-device
    DVE cast, but half the DMA bytes and no cast latency.
    """
    # x -> [B, E, P, KT*L]: [b, e, p, k*L+l] = x[b, e*C + k*128 + p, l]
    x16 = (x.astype(np.float16)
            .reshape(B, E, KT, P, L).transpose(0, 1, 3, 2, 4)
            .reshape(B, E, P, KT * L))
    # W1T[e] = W1r[e].T -> [E, C, F] (rows are k*128+p, already contiguous)
    w1t = np.ascontiguousarray(
        W1.astype(np.float32).reshape(E, F, C).transpose(0, 2, 1)
    ).astype(np.float16)
    # W2 -> [E, P, QT*C]: [e, p, q*C+c] = W2T[e, q*128+p, c]
    w2t = (W2.astype(np.float32).reshape(E, C, F).transpose(0, 2, 1)
             .reshape(E, QT, P, C).transpose(0, 2, 1, 3)
             .reshape(E, P, QT * C)).astype(np.float16)
    # biases -> [P, E*MT] / [P, E*JT]: [p, e*MT+m] = b1[e, m*128+p]
    b1r = (b1.astype(np.float32).reshape(E, MT, P)
             .transpose(2, 0, 1).reshape(P, E * MT))
    b2r = (b2.astype(np.float32).reshape(E, JT, P)
             .transpose(2, 0, 1).reshape(P, E * JT))
    in_maps = []
    for i in range(NCORES):
        es = slice(i * EPC, (i + 1) * EPC)
        b1c = b1r.reshape(P, E, MT)[:, es].reshape(P, EPC * MT)
        b2c = b2r.reshape(P, E, JT)[:, es].reshape(P, EPC * JT)
        in_maps.append({
            "xs": np.ascontiguousarray(x16[:, es]),
            "w1t": np.ascontiguousarray(w1t[es]),
            "b1s": np.ascontiguousarray(b1c),
            "w2t": np.ascontiguousarray(w2t[es]),
            "b2s": np.ascontiguousarray(b2c),
        })
    return in_maps


def run(x, W1, b1, W2, b2, trace=False, **trace_kwargs):
    nc = _get_nc()
    in_maps = _shard_inputs(x, W1, b1, W2, b2)
    res = run_bass_kernel_spmd(
        nc, in_maps, core_ids=list(range(NCORES)), trace=trace, **trace_kwargs
    )
    y = np.concatenate([res.results[i]["ys"] for i in range(NCORES)], axis=1)
    return y, res


def kernel(x, W1, b1, W2, b2):
    y, _ = run(x, W1, b1, W2, b2)
    return y.astype(np.float32)


# revision 12
# speedup vs baseline: 1.1828x; 1.0040x over previous
"""Expert-parallel grouped-MLP (MoE experts) kernel for 8 Trainium2 cores.

Problem: y = W2_e @ silu(W1_e @ x_e + b1_e) + b2_e for E=16 independent
experts (grouped 1x1 conv), B=8 batches, C=256 channels/expert, CAP=4,
L=1024 positions.

Sharding: expert-parallel — core i owns experts {2i, 2i+1}; no cross-core
communication. Host pre-transposes weights into lhsT layout and pre-casts
x/W1/W2 to fp16 (same numerics as an on-device DVE cast, but half the
DMA bytes and no cast on the startup critical path).

Per (b, e) pair on-device:
  layer 1: 8 m-tiles x (2k x 2n) fp16 matmuls -> PSUM[128,1024]
           ACT silu(. + b1) PSUM -> h SBUF [128, 8x1024] (fp16)
  layer 2: 2 j-tiles x (8q x 2n) accumulating matmuls -> PSUM[128,1024]
           DVE + b2 PSUM -> y SBUF fp32, DMA out

DMA-trigger scheduling: a dma_start costs ~0.6-0.8us of sequencer time,
so triggers are coalesced into few multi-dim-AP descriptors (one DMA per
expert for W1/W2, one per pair for x, one per bias tensor) and split
across queues: GpSimd carries all x loads (first trigger ~6.5us -> pair-0
x complete ~8.5us), Sync carries weights/biases and the y stores
(gpsimd-SWDGE stores lengthened the teardown by ~3us when tried).
"""
import numpy as np

import concourse.tile as tile
from concourse import bacc, mybir
from concourse.bass_utils import run_bass_kernel_spmd

# Problem constants (hardcoded per contract)
B, E, C, CAP, L = 8, 16, 256, 4, 1024
F = C * CAP            # 1024 hidden per expert
NCORES = 8
EPC = E // NCORES      # 2 experts per core
P = 128                # partitions
KT = C // P            # 2 k-tiles (layer-1 contraction)
MT = F // P            # 8 m-tiles (layer-1 output partitions)
JT = C // P            # 2 j-tiles (layer-2 output partitions)
QT = F // P            # 8 q-tiles (layer-2 contraction)
NT = L // 512          # 2 n-tiles of 512 cols
N_WARMUP = 5           # dummy PE warmup matmuls (bridge engine-boot -> data)

_FP32 = mybir.dt.float32
_FP16 = mybir.dt.float16


def _build():
    nc = bacc.Bacc("TRN2", target_bir_lowering=False, debug=False)

    # All inputs are host-prearranged into SBUF layout so every input DMA
    # is a contiguous [128, N] copy (descriptor processing cost on the
    # trigger queue scales with segment count — a gathered bias AP cost
    # 7.7us of sync-queue time when tried).
    xs_d = nc.dram_tensor("xs", [B, EPC, P, KT * L], _FP16, kind="ExternalInput")
    w1t_d = nc.dram_tensor("w1t", [EPC, C, F], _FP16, kind="ExternalInput")
    b1s_d = nc.dram_tensor("b1s", [P, EPC * MT], _FP32, kind="ExternalInput")
    w2t_d = nc.dram_tensor("w2t", [EPC, P, QT * C], _FP16, kind="ExternalInput")
    b2s_d = nc.dram_tensor("b2s", [P, EPC * JT], _FP32, kind="ExternalInput")
    ys_d = nc.dram_tensor("ys", [B, EPC * C, L], _FP32, kind="ExternalOutput")

    with tile.TileContext(nc) as tc:
        with (
            tc.tile_pool(name="const", bufs=1) as cpool,
            tc.tile_pool(name="x", bufs=6) as xpool,
            tc.tile_pool(name="h", bufs=2) as hpool,
            tc.tile_pool(name="y", bufs=3) as ypool,
            tc.tile_pool(name="ps", bufs=8, space="PSUM") as pspool,
        ):
            # ---- PE warmup: zero bf16 matmuls; memsets on Vector (the
            # GpSimd queue is reserved for the x-load DMA triggers) ----
            wdum = cpool.tile([P, P], mybir.dt.bfloat16, tag="wdum")
            rdum = cpool.tile([P, 512], mybir.dt.bfloat16, tag="rdum")
            bdum = cpool.tile([P, 1], _FP32, tag="bdum")
            nc.vector.memset(wdum[:], 0.0)
            nc.vector.memset(rdum[:], 0.0)
            nc.vector.memset(bdum[:], 0.0)
            actdum = cpool.tile([P, 1], _FP32, tag="actdum")
            # AP bias (not immediate) so the ACT table config matches the
            # real silus and the table is not reloaded mid-stream
            nc.scalar.activation(actdum[:], rdum[:, :1],
                                 mybir.ActivationFunctionType.Silu,
                                 bias=bdum[:, 0:1])
            for i in range(N_WARMUP):
                pdum = pspool.tile([P, 512], _FP32, tag="ps")
                nc.tensor.matmul(pdum[:], wdum[:], rdum[:],
                                 start=True, stop=True)

            # ---- weight/bias tiles (fp16 direct from DRAM, no casts) ----
            # w1sb[e][k]: [128, F];   [p, f] = W1T[e, k*128+p, f]
            # w2sb[e]:    [128, QT*C]; [p, q*C+c] = W2T[e, q*128+p, c]
            w1sb = [[cpool.tile([P, F], _FP16, tag=f"w1_{e}_{k}",
                                name=f"w1sb_{e}_{k}")
                     for k in range(KT)] for e in range(EPC)]
            w2sb = [cpool.tile([P, QT * C], _FP16, tag=f"w2_{e}",
                               name=f"w2sb_{e}")
                    for e in range(EPC)]
            b1sb = cpool.tile([P, EPC * MT], _FP32, tag="b1")  # col e*MT+m
            b2sb = cpool.tile([P, EPC * JT], _FP32, tag="b2")  # col e*JT+j

            def load_w1(e, k, eng):
                eng.dma_start(
                    w1sb[e][k][:],
                    w1t_d.ap()[e, k * P:(k + 1) * P, :],
                )

            def load_w2(e, eng):
                # one contiguous DMA per expert (host-prearranged layout)
                eng.dma_start(w2sb[e][:], w2t_d.ap()[e])

            def load_biases(eng):
                eng.dma_start(b1sb[:], b1s_d.ap()[:, :])
                eng.dma_start(b2sb[:], b2s_d.ap()[:, :])

            def load_x(b, e):
                # one contiguous 512KB DMA per pair (host-prearranged)
                xt = xpool.tile([P, KT * L], _FP16, tag="x",
                                name=f"x_{b}_{e}")
                nc.gpsimd.dma_start(xt[:], xs_d.ap()[b, e])
                return xt

            # ---- startup-critical DMAs ----
            # ALL pair-0 data on the sync queue: sync-triggered DMAs ride
            # the hardware DGE rings and land ~2-4us sooner than
            # gpsimd-SWDGE ones. x(0,0) as two k-half DMAs interleaved
            # with the W1 tiles in consumption order.
            x00 = [cpool.tile([P, L], _FP16, tag=f"x00_{k}",
                              name=f"x00_{k}") for k in range(KT)]
            nc.sync.dma_start(x00[0][:], xs_d.ap()[0, 0, :, 0:L])
            load_w1(0, 0, nc.sync)
            nc.sync.dma_start(x00[1][:], xs_d.ap()[0, 0, :, L:2 * L])
            load_w1(0, 1, nc.sync)
            # keep the sync queue SHORT before the first matmuls; biases
            # and W2 ride the scalar queue (idle until the first silu)
            load_biases(nc.scalar)
            load_w2(0, nc.scalar)

            # ---- per-(expert, batch) pipeline ----
            for e in range(EPC):
                for b in range(B):
                    first_pair = (e == 0 and b == 0)
                    last_pair = (e == EPC - 1 and b == B - 1)
                    xsb = None if first_pair else load_x(b, e)
                    if e == 0 and b == 3:
                        # expert-1 weights (needed at pair 8, ~120us in).
                        # Deliberately LATE in program order: consumers wait
                        # on cumulative per-queue DMA counts, so any extra
                        # early sync-queue DMA delays pair-0's first matmul.
                        load_w1(1, 0, nc.sync)
                        load_w1(1, 1, nc.sync)
                        load_w2(1, nc.sync)

                    # layer 1: h = silu(W1 @ x + b1), h[p, m*L + l]
                    hsb = hpool.tile([P, MT * L], _FP16, tag="h")
                    for m in range(MT):
                        # two 1-bank psum tiles per m-tile: finer slot
                        # recycling so the m-loop never stalls on silus
                        psh = [pspool.tile([P, 512], _FP32, tag="ps",
                                           name=f"psh_{e}_{b}_{m}_{n}")
                               for n in range(NT)]
                        for k in range(KT):
                            rhs_t = x00[k] if first_pair else xsb
                            off = 0 if first_pair else k * L
                            for n in range(NT):
                                nc.tensor.matmul(
                                    psh[n][:],
                                    w1sb[e][k][:, m * P:(m + 1) * P],
                                    rhs_t[:, off + n * 512:
                                            off + (n + 1) * 512],
                                    start=(k == 0),
                                    stop=(k == KT - 1),
                                )
                        for n in range(NT):
                            nc.scalar.activation(
                                hsb[:, m * L + n * 512:
                                      m * L + (n + 1) * 512],
                                psh[n][:],
                                mybir.ActivationFunctionType.Silu,
                                bias=b1sb[:, e * MT + m: e * MT + m + 1],
                            )

                    # layer 2: y = W2 @ h + b2
                    for j in range(JT):
                        if last_pair:
                            # separate 1-bank psum per n so DVE(n0) overlaps
                            # the n1 matmul chain; store triggered from the
                            # Vector queue to keep the tail short
                            for n in range(NT):
                                psn = pspool.tile([P, 512], _FP32, tag="ps",
                                                  name=f"psn_{j}_{n}")
                                for q in range(QT):
                                    nc.tensor.matmul(
                                        psn[:],
                                        w2sb[e][:, q * C + j * P:
                                                q * C + (j + 1) * P],
                                        hsb[:, q * L + n * 512:
                                              q * L + (n + 1) * 512],
                                        start=(q == 0),
                                        stop=(q == QT - 1),
                                    )
                                ysn = ypool.tile([P, 512], _FP32, tag="y",
                                                 name=f"ysn_{j}_{n}")
                                nc.vector.tensor_scalar_add(
                                    ysn[:],
                                    psn[:],
                                    b2sb[:, e * JT + j: e * JT + j + 1],
                                )
                                nc.sync.dma_start(
                                    ys_d.ap()[b,
                                              e * C + j * P: e * C + (j + 1) * P,
                                              n * 512:(n + 1) * 512],
                                    ysn[:],
                                )
                            continue
                        psy = [pspool.tile([P, 512], _FP32, tag="ps",
                                           name=f"psy_{e}_{b}_{j}_{n}")
                               for n in range(NT)]
                        for q in range(QT):
                            for n in range(NT):
                                nc.tensor.matmul(
                                    psy[n][:],
                                    w2sb[e][:, q * C + j * P: q * C + (j + 1) * P],
                                    hsb[:, q * L + n * 512: q * L + (n + 1) * 512],
                                    start=(q == 0),
                                    stop=(q == QT - 1),
                                )
                        ysb = ypool.tile([P, L], _FP32, tag="y",
                                         name=f"ysb_{e}_{b}_{j}")
                        for n in range(NT):
                            nc.vector.tensor_scalar_add(
                                ysb[:, n * 512:(n + 1) * 512],
                                psy[n][:],
                                b2sb[:, e * JT + j: e * JT + j + 1],
                            )
                        nc.sync.dma_start(
                            ys_d.ap()[b, e * C + j * P: e * C + (j + 1) * P, :],
                            ysb[:],
                        )

    nc.compile()
    return nc


_NC_CACHE = None


def _get_nc():
    global _NC_CACHE
    if _NC_CACHE is None:
        _NC_CACHE = _build()
    return _NC_CACHE


def _shard_inputs(x, W1, b1, W2, b2):
    """Full inputs -> list of 8 per-core input dicts (expert-parallel).

    Host pre-casts x/W1/W2 to fp16 — identical numerics to an on-device
    DVE cast, but half the DMA bytes and no cast latency.
    """
    # x -> [B, E, P, KT*L]: [b, e, p, k*L+l] = x[b, e*C + k*128 + p, l]
    x16 = (x.astype(np.float16)
            .reshape(B, E, KT, P, L).transpose(0, 1, 3, 2, 4)
            .reshape(B, E, P, KT * L))
    # W1T[e] = W1r[e].T -> [E, C, F] (rows are k*128+p, already contiguous)
    w1t = np.ascontiguousarray(
        W1.astype(np.float32).reshape(E, F, C).transpose(0, 2, 1)
    ).astype(np.float16)
    # W2 -> [E, P, QT*C]: [e, p, q*C+c] = W2T[e, q*128+p, c]
    w2t = (W2.astype(np.float32).reshape(E, C, F).transpose(0, 2, 1)
             .reshape(E, QT, P, C).transpose(0, 2, 1, 3)
             .reshape(E, P, QT * C)).astype(np.float16)
    # biases -> [P, E*MT] / [P, E*JT]: [p, e*MT+m] = b1[e, m*128+p]
    b1r = (b1.astype(np.float32).reshape(E, MT, P)
             .transpose(2, 0, 1).reshape(P, E * MT))
    b2r = (b2.astype(np.float32).reshape(E, JT, P)
             .transpose(2, 0, 1).reshape(P, E * JT))
    in_maps = []
    for i in range(NCORES):
        es = slice(i * EPC, (i + 1) * EPC)
        b1c = b1r.reshape(P, E, MT)[:, es].reshape(P, EPC * MT)
        b2c = b2r.reshape(P, E, JT)[:, es].reshape(P, EPC * JT)
        in_maps.append({
            "xs": np.ascontiguousarray(x16[:, es]),
            "w1t": np.ascontiguousarray(w1t[es]),
            "b1s": np.ascontiguousarray(b1c),
            "w2t": np.ascontiguousarray(w2t[es]),
            "b2s": np.ascontiguousarray(b2c),
        })
    return in_maps


def run(x, W1, b1, W2, b2, trace=False, **trace_kwargs):
    nc = _get_nc()
    in_maps = _shard_inputs(x, W1, b1, W2, b2)
    res = run_bass_kernel_spmd(
        nc, in_maps, core_ids=list(range(NCORES)), trace=trace, **trace_kwargs
    )
    y = np.concatenate([res.results[i]["ys"] for i in range(NCORES)], axis=1)
    return y, res


def kernel(x, W1, b1, W2, b2):
    y, _ = run(x, W1, b1, W2, b2)
    return y.astype(np.float32)


# revision 13
# speedup vs baseline: 1.2170x; 1.0289x over previous
"""Expert-parallel grouped-MLP (MoE experts) kernel for 8 Trainium2 cores.

Problem: y = W2_e @ silu(W1_e @ x_e + b1_e) + b2_e for E=16 independent
experts (grouped 1x1 conv), B=8 batches, C=256 channels/expert, CAP=4,
L=1024 positions.

Sharding: expert-parallel — core i owns experts {2i, 2i+1}; no cross-core
communication. Host pre-transposes weights into lhsT layout and pre-casts
x/W1/W2 to fp16 (same numerics as an on-device DVE cast, but half the
DMA bytes and no cast on the startup critical path).

Per (b, e) pair on-device:
  layer 1: 8 m-tiles x (2k x 2n) fp16 matmuls -> PSUM[128,1024]
           ACT silu(. + b1) PSUM -> h SBUF [128, 8x1024] (fp16)
  layer 2: 2 j-tiles x (8q x 2n) accumulating matmuls -> PSUM[128,1024]
           DVE + b2 PSUM -> y SBUF fp32, DMA out

DMA-trigger scheduling: a dma_start costs ~0.6-0.8us of sequencer time,
so triggers are coalesced into few multi-dim-AP descriptors (one DMA per
expert for W1/W2, one per pair for x, one per bias tensor) and split
across queues: GpSimd carries all x loads (first trigger ~6.5us -> pair-0
x complete ~8.5us), Sync carries weights/biases and the y stores
(gpsimd-SWDGE stores lengthened the teardown by ~3us when tried).
"""
import numpy as np

import concourse.tile as tile
from concourse import bacc, mybir
from concourse.bass_utils import run_bass_kernel_spmd

# Problem constants (hardcoded per contract)
B, E, C, CAP, L = 8, 16, 256, 4, 1024
F = C * CAP            # 1024 hidden per expert
NCORES = 8
EPC = E // NCORES      # 2 experts per core
P = 128                # partitions
KT = C // P            # 2 k-tiles (layer-1 contraction)
MT = F // P            # 8 m-tiles (layer-1 output partitions)
JT = C // P            # 2 j-tiles (layer-2 output partitions)
QT = F // P            # 8 q-tiles (layer-2 contraction)
NT = L // 512          # 2 n-tiles of 512 cols
N_WARMUP = 3           # dummy PE warmup matmuls (bridge engine-boot -> data)

_FP32 = mybir.dt.float32
_FP16 = mybir.dt.float16


def _build():
    nc = bacc.Bacc("TRN2", target_bir_lowering=False, debug=False)

    # All inputs are host-prearranged into SBUF layout so every input DMA
    # is a contiguous [128, N] copy (descriptor processing cost on the
    # trigger queue scales with segment count — a gathered bias AP cost
    # 7.7us of sync-queue time when tried).
    xs_d = nc.dram_tensor("xs", [B, EPC, P, KT * L], _FP16, kind="ExternalInput")
    w1t_d = nc.dram_tensor("w1t", [EPC, C, F], _FP16, kind="ExternalInput")
    b1s_d = nc.dram_tensor("b1s", [P, EPC * MT], _FP32, kind="ExternalInput")
    w2t_d = nc.dram_tensor("w2t", [EPC, P, QT * C], _FP16, kind="ExternalInput")
    b2s_d = nc.dram_tensor("b2s", [P, EPC * JT], _FP32, kind="ExternalInput")
    ys_d = nc.dram_tensor("ys", [B, EPC * C, L], _FP32, kind="ExternalOutput")

    with tile.TileContext(nc) as tc:
        with (
            tc.tile_pool(name="const", bufs=1) as cpool,
            tc.tile_pool(name="x", bufs=4) as xpool,
            tc.tile_pool(name="h", bufs=2) as hpool,
            tc.tile_pool(name="y", bufs=3) as ypool,
            tc.tile_pool(name="ps", bufs=8, space="PSUM") as pspool,
        ):
            # ---- PE warmup: zero bf16 matmuls; memsets on Vector (the
            # GpSimd queue is reserved for the x-load DMA triggers) ----
            wdum = cpool.tile([P, P], mybir.dt.bfloat16, tag="wdum")
            rdum = cpool.tile([P, 512], mybir.dt.bfloat16, tag="rdum")
            bdum = cpool.tile([P, 1], _FP32, tag="bdum")
            nc.vector.memset(wdum[:], 0.0)
            nc.vector.memset(rdum[:], 0.0)
            nc.vector.memset(bdum[:], 0.0)
            actdum = cpool.tile([P, 1], _FP32, tag="actdum")
            # AP bias (not immediate) so the ACT table config matches the
            # real silus and the table is not reloaded mid-stream
            nc.scalar.activation(actdum[:], rdum[:, :1],
                                 mybir.ActivationFunctionType.Silu,
                                 bias=bdum[:, 0:1])
            for i in range(N_WARMUP):
                pdum = pspool.tile([P, 512], _FP32, tag="ps")
                nc.tensor.matmul(pdum[:], wdum[:], rdum[:],
                                 start=True, stop=True)

            # ---- weight/bias tiles (fp16 direct from DRAM, no casts) ----
            # w1sb[e][k]: [128, F];   [p, f] = W1T[e, k*128+p, f]
            # w2sb[e]:    [128, QT*C]; [p, q*C+c] = W2T[e, q*128+p, c]
            w1sb = [[cpool.tile([P, F], _FP16, tag=f"w1_{e}_{k}",
                                name=f"w1sb_{e}_{k}")
                     for k in range(KT)] for e in range(EPC)]
            w2sb = [cpool.tile([P, QT * C], _FP16, tag=f"w2_{e}",
                               name=f"w2sb_{e}")
                    for e in range(EPC)]
            b1sb = cpool.tile([P, EPC * MT], _FP32, tag="b1")  # col e*MT+m
            b2sb = cpool.tile([P, EPC * JT], _FP32, tag="b2")  # col e*JT+j

            def load_w1(e, k, eng):
                eng.dma_start(
                    w1sb[e][k][:],
                    w1t_d.ap()[e, k * P:(k + 1) * P, :],
                )

            def load_w2(e, eng):
                # one contiguous DMA per expert (host-prearranged layout)
                eng.dma_start(w2sb[e][:], w2t_d.ap()[e])

            def load_biases(eng):
                eng.dma_start(b1sb[:], b1s_d.ap()[:, :])
                eng.dma_start(b2sb[:], b2s_d.ap()[:, :])

            def load_x(b, e, gate=None):
                # one contiguous 512KB DMA per pair (host-prearranged).
                # `gate`: 1-element copy dependency so this DMA arms only
                # after the startup-critical transfers are through the DMA
                # engines (armed-descriptor FIFOs drain in arm order, so an
                # early prefetch would head-of-line-block pair-0's data).
                xt = xpool.tile([P, KT * L], _FP16, tag="x",
                                name=f"x_{b}_{e}")
                if gate is not None:
                    nc.gpsimd.tensor_copy(xt[0:1, 0:1], gate[0:1, 0:1])
                nc.gpsimd.dma_start(xt[:], xs_d.ap()[b, e])
                return xt

            # ---- startup-critical DMAs ----
            # ALL pair-0 data on the sync queue: sync-triggered DMAs ride
            # the hardware DGE rings and land ~2-4us sooner than
            # gpsimd-SWDGE ones. x(0,0) as two k-half DMAs interleaved
            # with the W1 tiles in consumption order.
            x00 = [cpool.tile([P, L], _FP16, tag=f"x00_{k}",
                              name=f"x00_{k}") for k in range(KT)]
            nc.sync.dma_start(x00[0][:], xs_d.ap()[0, 0, :, 0:L])
            load_w1(0, 0, nc.sync)
            nc.sync.dma_start(x00[1][:], xs_d.ap()[0, 0, :, L:2 * L])
            load_w1(0, 1, nc.sync)
            # keep the sync queue SHORT before the first matmuls; biases
            # and W2 ride the scalar queue (idle until the first silu)
            load_biases(nc.scalar)
            load_w2(0, nc.scalar)

            # ---- per-(expert, batch) pipeline ----
            for e in range(EPC):
                for b in range(B):
                    first_pair = (e == 0 and b == 0)
                    last_pair = (e == EPC - 1 and b == B - 1)
                    xsb = None if first_pair else load_x(b, e, gate=x00[1])
                    if e == 0 and b == 3:
                        # expert-1 weights (needed at pair 8, ~120us in).
                        # Deliberately LATE in program order: consumers wait
                        # on cumulative per-queue DMA counts, so any extra
                        # early sync-queue DMA delays pair-0's first matmul.
                        load_w1(1, 0, nc.sync)
                        load_w1(1, 1, nc.sync)
                        load_w2(1, nc.sync)

                    # layer 1: h = silu(W1 @ x + b1), h[p, m*L + l]
                    hsb = hpool.tile([P, MT * L], _FP16, tag="h")
                    for m in range(MT):
                        # two 1-bank psum tiles per m-tile: finer slot
                        # recycling so the m-loop never stalls on silus
                        psh = [pspool.tile([P, 512], _FP32, tag="ps",
                                           name=f"psh_{e}_{b}_{m}_{n}")
                               for n in range(NT)]
                        for k in range(KT):
                            rhs_t = x00[k] if first_pair else xsb
                            off = 0 if first_pair else k * L
                            for n in range(NT):
                                nc.tensor.matmul(
                                    psh[n][:],
                                    w1sb[e][k][:, m * P:(m + 1) * P],
                                    rhs_t[:, off + n * 512:
                                            off + (n + 1) * 512],
                                    start=(k == 0),
                                    stop=(k == KT - 1),
                                )
                        for n in range(NT):
                            nc.scalar.activation(
                                hsb[:, m * L + n * 512:
                                      m * L + (n + 1) * 512],
                                psh[n][:],
                                mybir.ActivationFunctionType.Silu,
                                bias=b1sb[:, e * MT + m: e * MT + m + 1],
                            )

                    # layer 2: y = W2 @ h + b2
                    for j in range(JT):
                        if last_pair:
                            # separate 1-bank psum per n so DVE(n0) overlaps
                            # the n1 matmul chain; store triggered from the
                            # Vector queue to keep the tail short
                            for n in range(NT):
                                psn = pspool.tile([P, 512], _FP32, tag="ps",
                                                  name=f"psn_{j}_{n}")
                                for q in range(QT):
                                    nc.tensor.matmul(
                                        psn[:],
                                        w2sb[e][:, q * C + j * P:
                                                q * C + (j + 1) * P],
                                        hsb[:, q * L + n * 512:
                                              q * L + (n + 1) * 512],
                                        start=(q == 0),
                                        stop=(q == QT - 1),
                                    )
                                ysn = ypool.tile([P, 512], _FP32, tag="y",
                                                 name=f"ysn_{j}_{n}")
                                nc.vector.tensor_scalar_add(
                                    ysn[:],
                                    psn[:],
                                    b2sb[:, e * JT + j: e * JT + j + 1],
                                )
                                nc.sync.dma_start(
                                    ys_d.ap()[b,
                                              e * C + j * P: e * C + (j + 1) * P,
                                              n * 512:(n + 1) * 512],
                                    ysn[:],
                                )
                            continue
                        psy = [pspool.tile([P, 512], _FP32, tag="ps",
                                           name=f"psy_{e}_{b}_{j}_{n}")
                               for n in range(NT)]
                        for q in range(QT):
                            for n in range(NT):
                                nc.tensor.matmul(
                                    psy[n][:],
                                    w2sb[e][:, q * C + j * P: q * C + (j + 1) * P],
                                    hsb[:, q * L + n * 512: q * L + (n + 1) * 512],
                                    start=(q == 0),
                                    stop=(q == QT - 1),
                                )
                        ysb = ypool.tile([P, L], _FP32, tag="y",
                                         name=f"ysb_{e}_{b}_{j}")
                        for n in range(NT):
                            nc.vector.tensor_scalar_add(
                                ysb[:, n * 512:(n + 1) * 512],
                                psy[n][:],
                                b2sb[:, e * JT + j: e * JT + j + 1],
                            )
                        nc.sync.dma_start(
                            ys_d.ap()[b, e * C + j * P: e * C + (j + 1) * P, :],
                            ysb[:],
                        )

    nc.compile()
    return nc


_NC_CACHE = None


def _get_nc():
    global _NC_CACHE
    if _NC_CACHE is None:
        _NC_CACHE = _build()
    return _NC_CACHE


def _shard_inputs(x, W1, b1, W2, b2):
    """Full inputs -> list of 8 per-core input dicts (expert-parallel).

    Host pre-casts x/W1/W2 to fp16 — identical numerics to an on-device
    DVE cast, but half the DMA bytes and no cast latency.
    """
    # x -> [B, E, P, KT*L]: [b, e, p, k*L+l] = x[b, e*C + k*128 + p, l]
    x16 = (x.astype(np.float16)
            .reshape(B, E, KT, P, L).transpose(0, 1, 3, 2, 4)
            .reshape(B, E, P, KT * L))
    # W1T[e] = W1r[e].T -> [E, C, F] (rows are k*128+p, already contiguous)
    w1t = np.ascontiguousarray(
        W1.astype(np.float32).reshape(E, F, C).transpose(0, 2, 1)
    ).astype(np.float16)
    # W2 -> [E, P, QT*C]: [e, p, q*C+c] = W2T[e, q*128+p, c]
    w2t = (W2.astype(np.float32).reshape(E, C, F).transpose(0, 2, 1)
             .reshape(E, QT, P, C).transpose(0, 2, 1, 3)
             .reshape(E, P, QT * C)).astype(np.float16)
    # biases -> [P, E*MT] / [P, E*JT]: [p, e*MT+m] = b1[e, m*128+p]
    b1r = (b1.astype(np.float32).reshape(E, MT, P)
             .transpose(2, 0, 1).reshape(P, E * MT))
    b2r = (b2.astype(np.float32).reshape(E, JT, P)
             .transpose(2, 0, 1).reshape(P, E * JT))
    in_maps = []
    for i in range(NCORES):
        es = slice(i * EPC, (i + 1) * EPC)
        b1c = b1r.reshape(P, E, MT)[:, es].reshape(P, EPC * MT)
        b2c = b2r.reshape(P, E, JT)[:, es].reshape(P, EPC * JT)
        in_maps.append({
            "xs": np.ascontiguousarray(x16[:, es]),
            "w1t": np.ascontiguousarray(w1t[es]),
            "b1s": np.ascontiguousarray(b1c),
            "w2t": np.ascontiguousarray(w2t[es]),
            "b2s": np.ascontiguousarray(b2c),
        })
    return in_maps


def run(x, W1, b1, W2, b2, trace=False, **trace_kwargs):
    nc = _get_nc()
    in_maps = _shard_inputs(x, W1, b1, W2, b2)
    res = run_bass_kernel_spmd(
        nc, in_maps, core_ids=list(range(NCORES)), trace=trace, **trace_kwargs
    )
    y = np.concatenate([res.results[i]["ys"] for i in range(NCORES)], axis=1)
    return y, res


def kernel(x, W1, b1, W2, b2):
    y, _ = run(x, W1, b1, W2, b2)
    return y.astype(np.float32)
